# revision 13
# baseline (speedup 1.0000x reference)
"""Trainium2 Bass kernel for nn_Decoder_644245094507 (sampling decoder).

Data-parallel across 8 NeuronCores: batch 2048 -> 256 per core.
Activations live transposed in SBUF: [hidden -> partitions(128) x chunks,
batch -> free(256)].

Algebraic restructuring (validated numerically against the jax reference):
every input-side matmul of the LSTMs and all 'ds' projection matmuls are
folded into per-class gather tables applied with exact one-hot matmuls:
  CE   = c_emb @ c_ds_W[:, :H].T            [64, H]
  ME   = m_emb @ m_ds_W[:, :H].T            [128, H]
  A    = m_Wih @ m_ds_W[:, H:2H]            [4H, H]
  WCE  = c_Wih @ CE.T    (chord LSTM input gates, gathered by cidx_{t-1})
  WME  = m_Wih @ ME.T    (note  LSTM input gates, gathered by nidx_prev)
  ACE  = A @ CE.T        (ce-dependent part of note gates, by cidx)
  WmCE = m_Wih @ CE.T    (note-0 extra term, by cidx_t)
plus per-batch constants computed on device in a one-time preamble:
  Zc = z @ c_ds_W[:,H:].T + c_ds_b ; Zm = z @ m_ds_W[:,2H:].T + m_ds_b
  S = Zc @ m_ds_W[:,H:2H].T + Zm
  GZc = c_Wih @ Zc.T + cbias ; C1 = m_Wih @ S.T + mbias
  C0x = m_Wih @ Zc.T         ; Gz = c_Wih @ z.T + cbias
Per step only the hidden-state matmuls (Whh, p1, p2) and the gather matmuls
remain. fp32 PE matmuls (exact); ACT used only for Tanh
(sigmoid(x) = 0.5*tanh(x/2)+0.5); relu/bias on the vector engine.
"""
import numpy as np
from contextlib import ExitStack

import concourse.bass as bass
import concourse.tile as tile
from concourse import bacc, mybir
from concourse.bass_utils import run_bass_kernel_spmd

F32 = mybir.dt.float32
F16 = mybir.dt.float16
AF = mybir.ActivationFunctionType
OP = mybir.AluOpType

H = 512
CPL = 64          # chord classes
MPL = 128         # note classes
NOTES = 8
NCORES = 8
B_FULL = 2048
BC = B_FULL // NCORES   # 256 batch per core
KC = H // 128           # 4 k-chunks
MC = 4 * H // 128       # 16 gate m-chunks

_CACHE = {}
LAST_RESULTS = None   # BassKernelResults of the most recent run (for profiling)

HEAD_DIMS = {'tempo': 1, 'key': 12, 'mode': 7, 'valence': 1, 'energy': 1}


def _host_fold(p):
    """Weight-only folding on host (fp64 accumulate, fp32 results)."""
    d = {k: np.asarray(v, dtype=np.float64) for k, v in p.items()}
    f = {}

    def t32(x):
        return np.ascontiguousarray(x, dtype=np.float32)

    c_Wb = d['c_ds_W'][:, H:]
    m_Wb = d['m_ds_W'][:, H:2 * H]; m_Wc = d['m_ds_W'][:, 2 * H:]
    CE = d['c_emb'] @ d['c_ds_W'][:, :H].T     # [64, H]
    ME = d['m_emb'] @ d['m_ds_W'][:, :H].T     # [128, H]
    A = d['m_Wih'] @ m_Wb                      # [4H, H]

    def split16(nm, x):
        hi = np.ascontiguousarray(x, dtype=np.float16)
        lo = np.ascontiguousarray(
            (np.asarray(x, dtype=np.float32) - hi.astype(np.float32)), dtype=np.float16)
        f[nm + '_h'] = hi; f[nm + '_l'] = lo

    split16('tab_wce', CE @ d['c_Wih'].T)      # [64, 4H]
    split16('tab_ace', CE @ A.T)               # [64, 4H]
    split16('tab_wme', ME @ d['m_Wih'].T)      # [128, 4H]
    split16('tab_wmce', CE @ d['m_Wih'].T)     # [64, 4H]
    split16('whh_c', d['c_Whh'].T)             # [512, 2048]
    split16('whh_m', d['m_Whh'].T)
    split16('p1w_c', d['c_p1_W'].T)            # [512, 512]
    split16('p1w_m', d['m_p1_W'].T)
    f['p2w_c'] = t32(d['c_p2_W'].T)     # [512, 64]
    f['p2w_m'] = t32(d['m_p2_W'].T)     # [512, 128]

    f['wih_c'] = t32(d['c_Wih'].T)      # [512, 2048]
    f['wih_m'] = t32(d['m_Wih'].T)      # [512, 2048]
    f['cdsbw'] = t32(c_Wb.T)            # [512, 512]
    f['mdscw'] = t32(m_Wc.T)            # [512, 512]
    f['mdsbw'] = t32(m_Wb.T)            # [512, 512]

    f['cbias'] = t32((d['c_bih'] + d['c_bhh']).reshape(MC, 128).T)   # [128,16]
    f['mbias'] = t32((d['m_bih'] + d['m_bhh']).reshape(MC, 128).T)
    f['p1b_c'] = t32(d['c_p1_b'].reshape(KC, 128).T)                 # [128,4]
    f['p1b_m'] = t32(d['m_p1_b'].reshape(KC, 128).T)
    f['cdsb'] = t32(d['c_ds_b'].reshape(KC, 128).T)
    f['mdsb'] = t32(d['m_ds_b'].reshape(KC, 128).T)
    f['p2row_c'] = t32(np.broadcast_to(d['c_p2_b'], (128, CPL)))
    f['p2row_m'] = t32(np.broadcast_to(d['m_p2_b'], (128, MPL)))
    f['ident'] = np.eye(128, dtype=np.float32)

    for nm in HEAD_DIMS:
        f[f'h_{nm}_w1'] = t32(d[nm + '_W1'].T)                       # [512, 256]
        f[f'h_{nm}_b1'] = t32(d[nm + '_b1'].reshape(2, 128).T)       # [128, 2]
        f[f'h_{nm}_w2'] = t32(d[nm + '_W2'].T)                       # [256, out]
        f[f'h_{nm}_b2'] = t32(d[nm + '_b2'].reshape(-1, 1))          # [out, 1]
    return f


def _build(num_chords):
    nc = bacc.Bacc(trn_type="TRN2")
    inp = {}

    def I(name, shape):
        inp[name] = nc.dram_tensor(name, list(shape), F32, kind="ExternalInput")
        return inp[name]

    I('zT', [H, BC])
    I('hxc', [H, BC]); I('cxc', [H, BC]); I('hxm', [H, BC]); I('cxm', [H, BC])
    for nm16, shape in (('tab_wce', [64, 4 * H]), ('tab_ace', [64, 4 * H]),
                        ('tab_wme', [128, 4 * H]), ('tab_wmce', [64, 4 * H]),
                        ('whh_c', [H, 4 * H]), ('whh_m', [H, 4 * H]),
                        ('p1w_c', [H, H]), ('p1w_m', [H, H])):
        for sfx in ('_h', '_l'):
            inp[nm16 + sfx] = nc.dram_tensor(nm16 + sfx, list(shape), F16,
                                             kind="ExternalInput")
    for nm, shape in (('p2w_c', [H, CPL]), ('p2w_m', [H, MPL]),
                      ('wih_c', [H, 4 * H]), ('wih_m', [H, 4 * H]),
                      ('cdsbw', [H, H]), ('mdscw', [H, H]), ('mdsbw', [H, H]),
                      ('cbias', [128, MC]), ('mbias', [128, MC]),
                      ('p1b_c', [128, KC]), ('p1b_m', [128, KC]),
                      ('cdsb', [128, KC]), ('mdsb', [128, KC]),
                      ('p2row_c', [128, CPL]), ('p2row_m', [128, MPL]),
                      ('ident', [128, 128])):
        I(nm, shape)
    for nm, od in HEAD_DIMS.items():
        I(f'h_{nm}_w1', [H, 256]); I(f'h_{nm}_b1', [128, 2])
        I(f'h_{nm}_w2', [256, od]); I(f'h_{nm}_b2', [od, 1])

    clogs_d = nc.dram_tensor('clogs', [num_chords, 2, 128, CPL], F32, kind="ExternalOutput")
    nlogs_d = nc.dram_tensor('nlogs', [num_chords, NOTES, 2, 128, MPL], F32, kind="ExternalOutput")
    heads_d = {nm: nc.dram_tensor(f'o_{nm}', [HEAD_DIMS[nm], BC], F32, kind="ExternalOutput")
               for nm in HEAD_DIMS}

    def r4(ap, x):   # [(k p), x] -> [p, k, x]
        return ap.rearrange("(k p) x -> p k x", p=128)

    with ExitStack() as ctx:
        tc = ctx.enter_context(tile.TileContext(nc))
        wp = ctx.enter_context(tc.tile_pool(name="wp", bufs=1))
        st = ctx.enter_context(tc.tile_pool(name="st", bufs=1))
        ps = ctx.enter_context(tc.tile_pool(name="ps", bufs=1, space="PSUM"))
        tp = ctx.enter_context(tc.tile_pool(name="tp", bufs=1))

        # ---------------- persistent loads ----------------
        W = {}
        for nm, shape in (('cbias', [128, MC]), ('mbias', [128, MC]),
                          ('p1b_c', [128, KC]), ('p1b_m', [128, KC]),
                          ('cdsb', [128, KC]), ('mdsb', [128, KC]),
                          ('p2row_c', [128, CPL]), ('p2row_m', [128, MPL]),
                          ('ident', [128, 128])):
            W[nm] = wp.tile(shape, F32, name=nm)
            nc.sync.dma_start(W[nm][:], inp[nm][:])

        hc = st.tile([128, KC, BC], F32, name="hc")
        nc.sync.dma_start(hc[:], r4(inp['hxc'], BC))
        cc = st.tile([128, KC, BC], F32, name="cc")
        nc.sync.dma_start(cc[:], r4(inp['cxc'], BC))
        hm = st.tile([128, KC, BC], F32, name="hm")
        nc.sync.dma_start(hm[:], r4(inp['hxm'], BC))
        cm = st.tile([128, KC, BC], F32, name="cm")
        nc.sync.dma_start(cm[:], r4(inp['cxm'], BC))

        Gm = st.tile([128, MC, BC], F32, name="Gm")   # per-chord const (Gz at t=0)
        dr = ctx.enter_context(tc.tile_pool(name="dr", bufs=1, space="DRAM"))
        C1b_d = dr.tile([MC, 128, BC], F32, name="C1b_d")
        C0x_d = dr.tile([MC, 128, BC], F32, name="C0x_d")
        GZc_d = dr.tile([MC, 128, BC], F32, name="GZc_d")

        def mm_acc(pt, lhsT_tile, rhs_tile, m0):
            for k in range(KC):
                nc.tensor.matmul(pt[:], lhsT_tile[:, k, m0:m0 + 128], rhs_tile[:, k, :],
                                 start=(k == 0), stop=(k == KC - 1))

        def mm3(pt, w_h, w_l, r_h, r_l, m0, close):
            # fp16-split product: W@x ~= Wh@xh + Wh@xl + Wl@xh, fp32 psum accum
            passes = ((w_h, r_h), (w_h, r_l), (w_l, r_h))
            for pi, (wt_, rt_) in enumerate(passes):
                for k in range(KC):
                    nc.tensor.matmul(pt[:], wt_[:, k, m0:m0 + 128], rt_[:, k, :],
                                     start=(pi == 0 and k == 0),
                                     stop=(close and pi == 2 and k == KC - 1))

        # ---------------- one-time preamble ----------------
        with tc.tile_pool(name="stg", bufs=1) as stg:
            zT = stg.tile([128, KC, BC], F32, name="zT")
            nc.sync.dma_start(zT[:], r4(inp['zT'], BC))

            # heads
            for nm, od in HEAD_DIMS.items():
                w1 = stg.tile([128, KC, 256], F32, tag="wstg", name=f"w1_{nm}")
                nc.sync.dma_start(w1[:], r4(inp[f'h_{nm}_w1'], 256))
                w2 = stg.tile([128, 2, od], F32, tag="hw2", name=f"w2_{nm}")
                nc.sync.dma_start(w2[:], inp[f'h_{nm}_w2'].rearrange("(k p) x -> p k x", p=128))
                b1 = stg.tile([128, 2], F32, tag="hb1", name=f"b1_{nm}")
                nc.sync.dma_start(b1[:], inp[f'h_{nm}_b1'][:])
                b2 = stg.tile([od, 1], F32, tag="hb2", name=f"b2_{nm}")
                nc.sync.dma_start(b2[:], inp[f'h_{nm}_b2'][:])
                aT = stg.tile([128, 2, BC], F32, tag="heada", name=f"aT_{nm}")
                for m in range(2):
                    pa = ps.tile([128, BC], F32, tag="p1", bufs=2, name=f"pa_{nm}{m}")
                    mm_acc(pa, w1, zT, m * 128)
                    nc.vector.tensor_scalar(aT[:, m, :], pa[:], b1[:, m:m + 1], 0.0,
                                            OP.add, OP.max)
                po = ps.tile([od, BC], F32, tag="p2", bufs=1, name=f"po_{nm}")
                for k in range(2):
                    nc.tensor.matmul(po[:], w2[:, k, :], aT[:, k, :],
                                     start=(k == 0), stop=(k == 1))
                ho = stg.tile([od, BC], F32, tag="heado", name=f"ho_{nm}")
                nc.vector.tensor_scalar(ho[:], po[:], b2[:, 0:1], None, OP.add)
                nc.sync.dma_start(heads_d[nm][:], ho[:])

            # Zc / Zm / S
            ZcT = stg.tile([128, KC, BC], F32, name="ZcT")
            ZmT = stg.tile([128, KC, BC], F32, name="ZmT")
            ST = stg.tile([128, KC, BC], F32, name="ST")
            for dst, wnm, bnm, rhs in ((ZcT, 'cdsbw', 'cdsb', zT),
                                       (ZmT, 'mdscw', 'mdsb', zT)):
                wt = stg.tile([128, KC, H], F32, tag="wstg", name=f"stg_{wnm}")
                nc.sync.dma_start(wt[:], r4(inp[wnm], H))
                for m in range(KC):
                    pz = ps.tile([128, BC], F32, tag="p1", bufs=2, name=f"pz_{wnm}{m}")
                    mm_acc(pz, wt, rhs, m * 128)
                    nc.vector.tensor_scalar(dst[:, m, :], pz[:], W[bnm][:, m:m + 1],
                                            None, OP.add)
            wtb = stg.tile([128, KC, H], F32, tag="wstg", name="stg_mdsbw")
            nc.sync.dma_start(wtb[:], r4(inp['mdsbw'], H))
            for m in range(KC):
                pz = ps.tile([128, BC], F32, tag="p1", bufs=2, name=f"pS{m}")
                mm_acc(pz, wtb, ZcT, m * 128)
                nc.vector.tensor_add(ST[:, m, :], pz[:], ZmT[:, m, :])

            # gate-space consts; wih staged in halves of 8 m-chunks
            for wnm, jobs in (('wih_m', ((C1b_d, ST, 'mbias'), (C0x_d, ZcT, None))),
                              ('wih_c', ((GZc_d, ZcT, 'cbias'), (Gm, zT, 'cbias')))):
                for hw in range(2):
                    wt = stg.tile([128, KC, 8 * 128], F32, tag="wstg", name=f"stg_{wnm}{hw}")
                    nc.sync.dma_start(wt[:], inp[wnm][:, hw * 1024:(hw + 1) * 1024]
                                      .rearrange("(k p) x -> p k x", p=128))
                    for dst, rhs, bnm in jobs:
                        for mm in range(8):
                            m = hw * 8 + mm
                            pg = ps.tile([128, BC], F32, tag="gates", bufs=4,
                                         name=f"pc_{wnm}_{m}_{0 if bnm else 1}")
                            mm_acc(pg, wt, rhs, mm * 128)
                            if dst is Gm:
                                nc.vector.tensor_scalar(dst[:, m, :], pg[:],
                                                        W[bnm][:, m:m + 1], None, OP.add)
                            else:
                                ob = stg.tile([128, BC], F32, tag="pre_out", bufs=4,
                                              name=f"ob_{wnm}_{m}_{0 if bnm else 1}")
                                if bnm is None:
                                    nc.vector.tensor_copy(ob[:], pg[:])
                                else:
                                    nc.vector.tensor_scalar(ob[:], pg[:],
                                                            W[bnm][:, m:m + 1], None, OP.add)
                                nc.sync.dma_start(dst[m], ob[:])

        # heavy runtime weights: load after preamble staging is released
        wh = ctx.enter_context(tc.tile_pool(name="wh", bufs=1))
        for nm16, shape in (('tab_wce', [64, 4 * H]), ('tab_ace', [64, 4 * H]),
                            ('tab_wme', [128, 4 * H]), ('tab_wmce', [64, 4 * H])):
            for sfx in ('_h', '_l'):
                W[nm16 + sfx] = wh.tile(shape, F16, name=nm16 + sfx)
                nc.sync.dma_start(W[nm16 + sfx][:], inp[nm16 + sfx][:])
        for nm16, wdt in (('whh_c', 4 * H), ('whh_m', 4 * H),
                          ('p1w_c', H), ('p1w_m', H)):
            for sfx in ('_h', '_l'):
                W[nm16 + sfx] = wh.tile([128, KC, wdt], F16, name=nm16 + sfx)
                nc.sync.dma_start(W[nm16 + sfx][:], r4(inp[nm16 + sfx], wdt))
        for nm, wdt in (('p2w_c', CPL), ('p2w_m', MPL)):
            W[nm] = wh.tile([128, KC, wdt], F32, name=nm)
            nc.sync.dma_start(W[nm][:], r4(inp[nm], wdt))

        # ---------------- recurrent loop ----------------
        # persistent one-hot carries (loop-uniform)
        OHCP = st.tile([CPL, BC], F16, name="OHCP")    # onehot(cidx_{t-1})
        OHNL = st.tile([MPL, BC], F16, name="OHNL")    # onehot of last note of chord t-1
        HC16 = (st.tile([128, KC, BC], F16, name="HC16H"),
                st.tile([128, KC, BC], F16, name="HC16L"))
        HM16 = (st.tile([128, KC, BC], F16, name="HM16H"),
                st.tile([128, KC, BC], F16, name="HM16L"))

        def split_h(h_state, pair, label):
            nc.vector.tensor_copy(pair[0][:], h_state[:])
            hl32 = tp.tile([128, KC, BC], F32, tag="hl32", bufs=2, name=f"hl32{label}")
            nc.vector.tensor_sub(hl32[:], h_state[:], pair[0][:])
            nc.vector.tensor_copy(pair[1][:], hl32[:])

        def lstm_tail(g4, h_state, c_state, pair, label):
            # g4: 4 gate-group tiles [128, KC, BC] (i, f, g, o); in-place.
            for gi, scl in ((0, 0.5), (1, 0.5), (2, 1.0), (3, 0.5)):
                nc.scalar.activation(g4[gi][:], g4[gi][:], AF.Tanh, scale=scl)
            for gi in (0, 1, 3):   # sigmoid = 0.5*tanh+0.5, in place
                nc.vector.tensor_scalar(g4[gi][:], g4[gi][:], 0.5, 0.5, OP.mult, OP.add)
            nc.vector.tensor_mul(g4[1][:], g4[1][:], c_state[:])     # sigf*c
            nc.vector.tensor_mul(g4[0][:], g4[0][:], g4[2][:])       # sigi*tanhg
            nc.vector.tensor_add(c_state[:], g4[1][:], g4[0][:])
            nc.scalar.activation(g4[2][:], c_state[:], AF.Tanh)      # tanh(c)
            nc.vector.tensor_mul(h_state[:], g4[3][:], g4[2][:])
            split_h(h_state, pair, label)

        def mlp_head(h16, p1wn, p1b, p2w, p2row, ncls, out_slices, label):
            h_h, h_l = h16
            a1 = tp.tile([128, KC, BC], F32, tag="a1", bufs=2, name=f"a1{label}")
            for m in range(KC):
                pp = ps.tile([128, BC], F32, tag="p1", bufs=2, name=f"pp{label}{m}")
                mm3(pp, W[p1wn + '_h'], W[p1wn + '_l'], h_h, h_l, m * 128, True)
                nc.vector.tensor_scalar(a1[:, m, :], pp[:], p1b[:, m:m + 1], 0.0,
                                        OP.add, OP.max)
            oht = tp.tile([ncls, BC], F16, tag=f"oht{ncls}", bufs=2, name=f"oht{label}")
            for half in range(2):
                pq = ps.tile([128, ncls], F32, tag="p2", bufs=1, name=f"pq{label}{half}")
                for k in range(KC):
                    nc.tensor.matmul(pq[:], a1[:, k, half * 128:(half + 1) * 128],
                                     p2w[:, k, :], start=(k == 0), stop=(k == KC - 1))
                logit = tp.tile([128, ncls], F32, tag="logit", bufs=3, name=f"lg{label}{half}")
                nc.vector.tensor_add(logit[:], pq[:], p2row[:])
                nc.sync.dma_start(out_slices[half], logit[:])
                mx = tp.tile([128, 8], F32, tag="mx", bufs=2, name=f"mx{label}{half}")
                nc.vector.max(mx[:], logit[:])
                oh = tp.tile([128, ncls], F32, tag="oh", bufs=2, name=f"oh{label}{half}")
                nc.vector.tensor_scalar(oh[:], logit[:], mx[:, 0:1], None, OP.is_equal)
                ptr = ps.tile([ncls, 128], F32, tag="tr", bufs=1, name=f"ptr{label}{half}")
                nc.tensor.transpose(ptr[:], oh[:], W['ident'][:])
                nc.vector.tensor_copy(oht[:, half * 128:(half + 1) * 128], ptr[:])
            return oht

        def chord_body(first, cslices, nslices, lb):
            """One chord + its 8 notes. first: python bool (chord 0).
            cslices: 2 dram APs for clog halves; nslices(n) -> 2 APs."""
            g4c = [tp.tile([128, KC, BC], F32, tag="g4", bufs=4, name=f"g4{lb}_{gi}")
                   for gi in range(4)]
            for m in range(MC):
                pg = ps.tile([128, BC], F32, tag="gates", bufs=4, name=f"pg{lb}{m}")
                mm3(pg, W['whh_c_h'], W['whh_c_l'], HC16[0], HC16[1], m * 128, first)
                if not first:
                    msl = slice(m * 128, (m + 1) * 128)
                    nc.tensor.matmul(pg[:], W['tab_wce_h'][:, msl], OHCP[:],
                                     start=False, stop=False)
                    nc.tensor.matmul(pg[:], W['tab_wce_l'][:, msl], OHCP[:],
                                     start=False, stop=True)
                gi, j = divmod(m, KC)
                if first:
                    nc.vector.tensor_add(g4c[gi][:, j, :], pg[:], Gm[:, m, :])
                else:
                    gz = tp.tile([128, BC], F32, tag="cstr", bufs=4, name=f"gz{lb}{m}")
                    nc.sync.dma_start(gz[:], GZc_d[m])
                    nc.vector.tensor_add(g4c[gi][:, j, :], pg[:], gz[:])
            lstm_tail(g4c, hc, cc, HC16, lb)
            ohc_cur = mlp_head(HC16, 'p1w_c', W['p1b_c'], W['p2w_c'], W['p2row_c'],
                               CPL, cslices, lb)

            # per-chord note const: Gm = ACE-gather(ohc_cur) + C1b
            for m in range(MC):
                pg = ps.tile([128, BC], F32, tag="gates", bufs=4, name=f"pgm{lb}{m}")
                msl = slice(m * 128, (m + 1) * 128)
                nc.tensor.matmul(pg[:], W['tab_ace_h'][:, msl], ohc_cur[:],
                                 start=True, stop=False)
                nc.tensor.matmul(pg[:], W['tab_ace_l'][:, msl], ohc_cur[:],
                                 start=False, stop=True)
                c1 = tp.tile([128, BC], F32, tag="cstr", bufs=4, name=f"c1{lb}{m}")
                nc.sync.dma_start(c1[:], C1b_d[m])
                nc.vector.tensor_add(Gm[:, m, :], pg[:], c1[:])

            ohn_prev = None
            for n in range(NOTES):
                nl = f"{lb}n{n}"
                g4 = [tp.tile([128, KC, BC], F32, tag="g4", bufs=4, name=f"g4{nl}_{gi}")
                      for gi in range(4)]
                for m in range(MC):
                    pg = ps.tile([128, BC], F32, tag="gates", bufs=4, name=f"pg{nl}{m}")
                    mm3(pg, W['whh_m_h'], W['whh_m_l'], HM16[0], HM16[1], m * 128, False)
                    msl = slice(m * 128, (m + 1) * 128)
                    if n == 0:
                        if not first:
                            nc.tensor.matmul(pg[:], W['tab_wme_h'][:, msl], OHNL[:],
                                             start=False, stop=False)
                            nc.tensor.matmul(pg[:], W['tab_wme_l'][:, msl], OHNL[:],
                                             start=False, stop=False)
                            nc.tensor.matmul(pg[:], W['tab_ace_h'][:, msl], OHCP[:],
                                             start=False, stop=False)
                            nc.tensor.matmul(pg[:], W['tab_ace_l'][:, msl], OHCP[:],
                                             start=False, stop=False)
                        nc.tensor.matmul(pg[:], W['tab_wmce_h'][:, msl], ohc_cur[:],
                                         start=False, stop=False)
                        nc.tensor.matmul(pg[:], W['tab_wmce_l'][:, msl], ohc_cur[:],
                                         start=False, stop=True)
                    else:
                        nc.tensor.matmul(pg[:], W['tab_wme_h'][:, msl], ohn_prev[:],
                                         start=False, stop=False)
                        nc.tensor.matmul(pg[:], W['tab_wme_l'][:, msl], ohn_prev[:],
                                         start=False, stop=True)
                    gi, j = divmod(m, KC)
                    if n == 0:
                        c0 = tp.tile([128, BC], F32, tag="cstr", bufs=4, name=f"c0{nl}{m}")
                        nc.sync.dma_start(c0[:], C0x_d[m])
                        if first:
                            nc.vector.scalar_tensor_tensor(
                                g4[gi][:, j, :], pg[:], W['mbias'][:, m:m + 1],
                                c0[:], OP.add, OP.add)
                        else:
                            c1n = tp.tile([128, BC], F32, tag="cstr", bufs=4,
                                          name=f"c1n{nl}{m}")
                            nc.sync.dma_start(c1n[:], C1b_d[m])
                            nc.vector.tensor_add(g4[gi][:, j, :], pg[:], c1n[:])
                            nc.vector.tensor_add(g4[gi][:, j, :], g4[gi][:, j, :],
                                                 c0[:])
                    else:
                        nc.vector.tensor_add(g4[gi][:, j, :], pg[:], Gm[:, m, :])
                lstm_tail(g4, hm, cm, HM16, nl)
                ohn_prev = mlp_head(HM16, 'p1w_m', W['p1b_m'], W['p2w_m'], W['p2row_m'],
                                    MPL, nslices(n), nl)
            # carry one-hots to next chord
            nc.vector.tensor_copy(OHCP[:], ohc_cur[:])
            nc.vector.tensor_copy(OHNL[:], ohn_prev[:])

        split_h(hc, HC16, "ini_c")
        split_h(hm, HM16, "ini_m")

        # chord 0 unrolled (uses Gz preloaded in Gm, no gathers from t-1)
        chord_body(True, [clogs_d[0, half] for half in range(2)],
                   lambda n: [nlogs_d[0, n, half] for half in range(2)], "c0")

        if num_chords > 1:
            ET = mybir.EngineType
            with tc.For_i(1, num_chords, hint_engines=(ET.PE, ET.DVE, ET.Activation)) as ti:
                def csl(half):
                    return clogs_d[bass.ds(ti, 1), half].rearrange("a p c -> (a p) c")

                def nsl(n):
                    return [nlogs_d[bass.ds(ti, 1), n, half].rearrange("a p c -> (a p) c")
                            for half in range(2)]
                chord_body(False, [csl(0), csl(1)], nsl, "cT")

    nc.finalize()
    return nc


def _axon_available():
    """True when this process can drive the 8 axon trn2 cores via PJRT."""
    try:
        import jax
        return len(jax.devices()) >= NCORES
    except Exception:
        return False


def kernel(z, hx_chords, cx_chords, hx_melody, cx_melody, params, num_chords):
    if not _axon_available():
        return _kernel_subprocess(z, hx_chords, cx_chords, hx_melody, cx_melody,
                                  params, num_chords)
    return _kernel_local(z, hx_chords, cx_chords, hx_melody, cx_melody,
                         params, num_chords)


def _kernel_subprocess(z, hx_chords, cx_chords, hx_melody, cx_melody, params,
                       num_chords):
    """Run the device part in a fresh process with the axon jax platform.

    Needed when the calling process already initialized jax on another
    platform (e.g. it ran the CPU reference first)."""
    import os
    import subprocess
    import sys
    import tempfile
    td = tempfile.mkdtemp(prefix="bass_decoder_")
    inp_path = os.path.join(td, "in.npz")
    out_path = os.path.join(td, "out.npz")
    save = {'z': np.asarray(z), 'hx_chords': np.asarray(hx_chords),
            'cx_chords': np.asarray(cx_chords), 'hx_melody': np.asarray(hx_melody),
            'cx_melody': np.asarray(cx_melody),
            'num_chords': np.int64(int(num_chords))}
    for k, v in params.items():
        save['p_' + k] = np.asarray(v)
    np.savez(inp_path, **save)
    boot = (
        "import numpy as np, importlib.util, sys\n"
        f"spec = importlib.util.spec_from_file_location('bass_decoder_kernel', {__file__!r})\n"
        "K = importlib.util.module_from_spec(spec); spec.loader.exec_module(K)\n"
        f"d = np.load({inp_path!r})\n"
        "params = {k[2:]: d[k] for k in d.files if k.startswith('p_')}\n"
        "out = K._kernel_local(d['z'], d['hx_chords'], d['cx_chords'],"
        " d['hx_melody'], d['cx_melody'], params, int(d['num_chords']))\n"
        f"np.savez({out_path!r}, **{{f'o{{i}}': o for i, o in enumerate(out)}})\n"
    )
    env = dict(os.environ)
    env['JAX_PLATFORMS'] = 'axon'
    subprocess.run([sys.executable, "-c", boot], check=True, env=env)
    d = np.load(out_path)
    return tuple(d[f'o{i}'] for i in range(7))


def _kernel_local(z, hx_chords, cx_chords, hx_melody, cx_melody, params, num_chords):
    num_chords = int(num_chords)
    z = np.asarray(z, dtype=np.float32)
    assert z.shape[0] == B_FULL, f"expected batch {B_FULL}, got {z.shape[0]}"

    if num_chords not in _CACHE:
        _CACHE[num_chords] = _build(num_chords)
    nc = _CACHE[num_chords]

    f = _host_fold({k: np.asarray(v) for k, v in params.items()})

    def T(x):
        return np.ascontiguousarray(np.asarray(x, dtype=np.float32).T)

    in_maps = []
    for c in range(NCORES):
        sl = slice(c * BC, (c + 1) * BC)
        m = dict(f)
        m['zT'] = T(z[sl])
        m['hxc'] = T(np.asarray(hx_chords)[sl]); m['cxc'] = T(np.asarray(cx_chords)[sl])
        m['hxm'] = T(np.asarray(hx_melody)[sl]); m['cxm'] = T(np.asarray(cx_melody)[sl])
        in_maps.append(m)

    import os
    trace = bool(int(os.environ.get('BASS_DECODER_TRACE', '0')))
    res = run_bass_kernel_spmd(nc, in_maps, core_ids=list(range(NCORES)), trace=trace)
    global LAST_RESULTS
    LAST_RESULTS = res

    chord_out = np.empty((B_FULL, num_chords, CPL), np.float32)
    note_out = np.empty((B_FULL, num_chords * NOTES, MPL), np.float32)
    heads = {nm: np.empty((B_FULL, od), np.float32) for nm, od in HEAD_DIMS.items()}
    for c in range(NCORES):
        r = res.results[c]
        sl = slice(c * BC, (c + 1) * BC)
        cl = r['clogs'].reshape(num_chords, BC, CPL)
        chord_out[sl] = np.transpose(cl, (1, 0, 2))
        nl = r['nlogs'].reshape(num_chords, NOTES, BC, MPL)
        note_out[sl] = np.transpose(nl, (2, 0, 1, 3)).reshape(BC, -1, MPL)
        for nm in HEAD_DIMS:
            heads[nm][sl] = r[f'o_{nm}'].T
    return (chord_out, note_out, heads['tempo'], heads['key'], heads['mode'],
            heads['valence'], heads['energy'])
